# revision 4
# baseline (speedup 1.0000x reference)
"""Two-layer LSTM (linear cell/output activations) + FC head on 8 NeuronCores.

Strategy (data-parallel over batch, per the sharding hint):
  - B=32 split across 8 cores -> B_local=4 per core; weights replicated.
  - All state kept transposed: h^T/c^T are [H on partitions, (k,b) on free],
    so the per-step recurrence matmul is  z^T = U^T @ h^T  with U as the
    stationary operand and gates landing as [128, 4] column blocks. All
    elementwise gate math then runs on full-partition [128, 8] tiles.
  - Input projections (x@W0+b0, h0@W1+b1) are batched per 64-step chunk so
    their weight loads amortize; only the U-recurrence runs step-by-step.
  - T=2048 processed by a hardware For_i loop over 32 chunks of 64 steps.
  - Matmul operands (weights, x, h) optionally bf16 (fp32 PSUM accumulate):
    fp32 matmuls cost 4 cycles/row on TRN2 and block fast-weight-load;
    bf16 halves the dominant per-step LDWEIGHTS cost. Cell state c and all
    gate math stay fp32.
"""

import os
import numpy as np
from contextlib import ExitStack

os.environ.setdefault("MYCRO_LOCAL_CACHE", "1")

B, T, I, H, O = 32, 2048, 128, 256, 128
NCORES = 8
BL = B // NCORES          # 4 batch elements per core
CHUNK = 64                # timesteps per loop body
G4 = 4 * H                # 1024 gate columns
NM = G4 // 128            # 8 gate chunks of 128
KT = H // 128             # 2 contraction tiles

MM_BF16 = True            # matmul operands in bf16

_cache = {}


def _np_mmdt():
    if MM_BF16:
        import ml_dtypes
        return ml_dtypes.bfloat16
    return np.float32


def _build(tiny=False, mm_bf16=None, repeat=None):
    import concourse.bacc as bacc
    import concourse.bass as bass
    import concourse.tile as tile
    import concourse.mybir as mybir

    if repeat is None:
        repeat = int(os.environ.get("BENCH_REPEAT", "1"))
    if mm_bf16 is None:
        mm_bf16 = MM_BF16
    f32 = mybir.dt.float32
    mdt = mybir.dt.bfloat16 if mm_bf16 else f32
    AF = mybir.ActivationFunctionType
    ALU = mybir.AluOpType

    nc = bacc.Bacc("TRN2", target_bir_lowering=False, debug=False,
                   num_devices=NCORES)

    xprep_d = nc.declare_dram_parameter("xprep", [I, BL, T], mdt, isOutput=False)
    w0_d = nc.declare_dram_parameter("w0", [I, G4], mdt, isOutput=False)
    u0_d = nc.declare_dram_parameter("u0", [H, G4], mdt, isOutput=False)
    w1_d = nc.declare_dram_parameter("w1", [H, G4], mdt, isOutput=False)
    u1_d = nc.declare_dram_parameter("u1", [H, G4], mdt, isOutput=False)
    wfc_d = nc.declare_dram_parameter("wfc", [H, O], mdt, isOutput=False)
    b0t_d = nc.declare_dram_parameter("b0t", [128, NM], f32, isOutput=False)
    b1t_d = nc.declare_dram_parameter("b1t", [128, NM], f32, isOutput=False)
    bfct_d = nc.declare_dram_parameter("bfct", [128, 1], f32, isOutput=False)
    out_d = nc.declare_dram_parameter("outT", [O, BL], f32, isOutput=True)

    with tile.TileContext(nc) as tc, ExitStack() as ctx:
        if tiny:
            pool = ctx.enter_context(tc.tile_pool(name="tp", bufs=1))
            t1 = pool.tile([128, BL], mdt, tag="t1")
            t2 = pool.tile([128, BL], f32, tag="t2")
            nc.sync.dma_start(t1[:, :], xprep_d[:, :, 0])
            nc.vector.tensor_copy(t2[:, :], t1[:, :])
            nc.sync.dma_start(out_d[:, :], t2[:, :])
            nc.compile()
            return nc

        const = ctx.enter_context(tc.tile_pool(name="const", bufs=1))
        work = ctx.enter_context(tc.tile_pool(name="work", bufs=3))
        psum = ctx.enter_context(tc.tile_pool(name="psum", bufs=2, space="PSUM"))

        # Persistent SBUF residents.
        xall = const.tile([128, BL * T], mdt, tag="xall")       # col = b*T + t
        w0 = const.tile([128, G4], mdt, tag="w0")
        u0 = [const.tile([128, G4], mdt, tag=f"u0_{k}", name=f"u0_{k}")
              for k in range(KT)]
        w1 = [const.tile([128, G4], mdt, tag=f"w1_{k}", name=f"w1_{k}")
              for k in range(KT)]
        u1 = [const.tile([128, G4], mdt, tag=f"u1_{k}", name=f"u1_{k}")
              for k in range(KT)]
        wf = [const.tile([128, O], mdt, tag=f"wf_{k}", name=f"wf_{k}")
              for k in range(KT)]
        b0t = const.tile([128, NM], f32, tag="b0t")
        b1t = const.tile([128, NM], f32, tag="b1t")
        bfct = const.tile([128, 1], f32, tag="bfct")
        # chunk buffers: col = k*(BL*CHUNK) + b*CHUNK + t   (h0t)
        #                col = m*(BL*CHUNK) + b*CHUNK + t   (xw0t/xw1t)
        h0t = const.tile([128, KT * BL * CHUNK], mdt, tag="h0t")
        xw0t = const.tile([128, NM * BL * CHUNK], f32, tag="xw0t")
        xw1t = const.tile([128, NM * BL * CHUNK], f32, tag="xw1t")
        # recurrent state, col = k*BL + b
        c0 = const.tile([128, KT * BL], f32, tag="c0")
        c1 = const.tile([128, KT * BL], f32, tag="c1")
        h1 = const.tile([128, KT * BL], mdt, tag="h1")

        nc.sync.dma_start(xall[:, :].rearrange("p (b t) -> p b t", b=BL),
                          xprep_d[:, :, :])
        nc.sync.dma_start(w0[:, :], w0_d[:, :])
        for k in range(KT):
            sl = slice(k * 128, (k + 1) * 128)
            nc.sync.dma_start(u0[k][:, :], u0_d[sl, :])
            nc.sync.dma_start(w1[k][:, :], w1_d[sl, :])
            nc.sync.dma_start(u1[k][:, :], u1_d[sl, :])
            nc.sync.dma_start(wf[k][:, :], wfc_d[sl, :])
        nc.sync.dma_start(b0t[:, :], b0t_d[:, :])
        nc.sync.dma_start(b1t[:, :], b1t_d[:, :])
        nc.sync.dma_start(bfct[:, :], bfct_d[:, :])

        nc.vector.memset(h0t[:, :], 0.0)
        nc.vector.memset(c0[:, :], 0.0)
        nc.vector.memset(c1[:, :], 0.0)
        nc.vector.memset(h1[:, :], 0.0)

        def lstm_step(tl, uw, xwt, cst, h_rhs_fn, h_out_ap):
            """One recurrence step. h_rhs_fn(k)->[128,BL] prev-h AP,
            h_out_ap: [128, KT, BL] target for the new h."""
            zp = psum.tile([128, NM * BL], f32, tag="zp")
            for m in range(NM):
                msl = slice(m * 128, (m + 1) * 128)
                for k in range(KT):
                    nc.tensor.matmul(zp[:, m * BL:(m + 1) * BL],
                                     lhsT=uw[k][:, msl], rhs=h_rhs_fn(k),
                                     start=(k == 0), stop=(k == KT - 1))
            zs = work.tile([128, NM * BL], f32, tag="zs")
            xw_ap = xwt[:, :].rearrange("p (m b t) -> p m b t",
                                        m=NM, b=BL)[:, :, :, tl]
            nc.vector.tensor_tensor(
                zs[:, :].rearrange("p (m b) -> p m b", m=NM),
                zp[:, :].rearrange("p (m b) -> p m b", m=NM),
                xw_ap, ALU.add)
            # gate cols after host-side permutation: i 0:S, f S:2S,
            # o 2S:3S, g 3S:4S -- one sigmoid covers i,f,o
            S = KT * BL
            nc.scalar.activation(zs[:, 0:3 * S], zs[:, 0:3 * S], AF.Sigmoid)
            ig = work.tile([128, KT * BL], f32, tag="ig")
            nc.vector.tensor_tensor(ig[:, :], zs[:, 0:S],
                                    zs[:, 3 * S:4 * S], ALU.mult)
            nc.vector.tensor_tensor(cst[:, :], zs[:, S:2 * S],
                                    cst[:, :], ALU.mult)
            nc.vector.tensor_tensor(cst[:, :], cst[:, :], ig[:, :], ALU.add)
            nc.vector.tensor_tensor(
                h_out_ap,
                zs[:, 2 * S:3 * S].rearrange("p (k b) -> p k b", k=KT),
                cst[:, :].rearrange("p (k b) -> p k b", k=KT), ALU.mult)

        h0t_4d = h0t[:, :].rearrange("p (k b t) -> p k b t", k=KT, b=BL)

        rep_ctx = tc.For_i(0, repeat, 1) if repeat > 1 else None
        if rep_ctx is not None:
            rep_ctx.__enter__()
        with tc.For_i(0, T, CHUNK) as iv:
            # stage this chunk's x columns (only dynamic access in the body)
            xq = work.tile([128, BL * CHUNK], mdt, tag="xq")
            nc.vector.tensor_copy(
                xq[:, :].rearrange("p (b t) -> p b t", b=BL),
                xall[:, :].rearrange("p (b t) -> p b t",
                                     b=BL)[:, :, bass.ds(iv, CHUNK)])
            # xw0 = x @ W0 + b0 for the chunk
            for m in range(NM):
                msl = slice(m * 128, (m + 1) * 128)
                csl = slice(m * BL * CHUNK, (m + 1) * BL * CHUNK)
                psx = psum.tile([128, BL * CHUNK], f32, tag="psx")
                nc.tensor.matmul(psx[:, :], lhsT=w0[:, msl], rhs=xq[:, :],
                                 start=True, stop=True)
                nc.scalar.activation(xw0t[:, csl], psx[:, :], AF.Identity,
                                     bias=b0t[:, m:m + 1])
            # layer-0 recurrence; h stream written into h0t
            for tl in range(CHUNK):
                pv = (tl - 1) % CHUNK
                lstm_step(
                    tl, u0, xw0t, c0,
                    lambda k: h0t_4d[:, k, :, pv],
                    h0t_4d[:, :, :, tl])
            # xw1 = h0 @ W1 + b1 for the chunk
            for m in range(NM):
                msl = slice(m * 128, (m + 1) * 128)
                csl = slice(m * BL * CHUNK, (m + 1) * BL * CHUNK)
                psx = psum.tile([128, BL * CHUNK], f32, tag="psx")
                for k in range(KT):
                    nc.tensor.matmul(
                        psx[:, :], lhsT=w1[k][:, msl],
                        rhs=h0t[:, k * BL * CHUNK:(k + 1) * BL * CHUNK],
                        start=(k == 0), stop=(k == KT - 1))
                nc.scalar.activation(xw1t[:, csl], psx[:, :], AF.Identity,
                                     bias=b1t[:, m:m + 1])
            # layer-1 recurrence; only final h kept
            for tl in range(CHUNK):
                lstm_step(
                    tl, u1, xw1t, c1,
                    lambda k: h1[:, k * BL:(k + 1) * BL],
                    h1[:, :].rearrange("p (k b) -> p k b", k=KT))

        if rep_ctx is not None:
            rep_ctx.__exit__(None, None, None)

        # FC head: out^T = Wfc^T @ h1^T + bfc
        psf = psum.tile([128, BL], f32, tag="psf")
        for k in range(KT):
            nc.tensor.matmul(psf[:, :], lhsT=wf[k][:, :],
                             rhs=h1[:, k * BL:(k + 1) * BL],
                             start=(k == 0), stop=(k == KT - 1))
        oT = work.tile([128, BL], f32, tag="oT")
        nc.scalar.activation(oT[:, :], psf[:, :], AF.Identity,
                             bias=bfct[:, 0:1])
        nc.sync.dma_start(out_d[:, :], oT[:, :])

    nc.compile()
    return nc


def _get_compiled():
    if "main" not in _cache:
        _cache["main"] = _build()
    return _cache["main"]


def _in_maps(input_seq, W0, U0, b0, W1, U1, b1, Wfc, bfc):
    mdt = _np_mmdt()
    x = np.asarray(input_seq, dtype=np.float32)
    # reorder gate blocks (i,f,g,o) -> (i,f,o,g) so one sigmoid instr
    # covers the first three
    perm = np.concatenate([np.arange(0, 2 * H),
                           np.arange(3 * H, 4 * H),
                           np.arange(2 * H, 3 * H)])

    def gp(w):
        return np.ascontiguousarray(
            np.asarray(w, np.float32)[..., perm].astype(mdt))

    shared = {
        "w0": gp(W0),
        "u0": gp(U0),
        "w1": gp(W1),
        "u1": gp(U1),
        "wfc": np.ascontiguousarray(np.asarray(Wfc, np.float32).astype(mdt)),
        "b0t": np.ascontiguousarray(
            np.asarray(b0, np.float32)[perm].reshape(NM, 128).T),
        "b1t": np.ascontiguousarray(
            np.asarray(b1, np.float32)[perm].reshape(NM, 128).T),
        "bfct": np.ascontiguousarray(np.asarray(bfc, np.float32).reshape(1, 128).T),
    }
    in_maps = []
    for c in range(NCORES):
        xs = x[c * BL:(c + 1) * BL]                       # [BL, T, I]
        xp = np.ascontiguousarray(xs.transpose(2, 0, 1).astype(mdt))
        m = dict(shared)
        m["xprep"] = xp
        in_maps.append(m)
    return in_maps


def _run(nc, inputs):
    from concourse.bass_utils import run_bass_kernel_spmd
    in_maps = _in_maps(**inputs)
    res = run_bass_kernel_spmd(nc, in_maps, list(range(NCORES)))
    out = np.empty((B, 1, O), np.float32)
    for c in range(NCORES):
        out[c * BL:(c + 1) * BL, 0, :] = res.results[c]["outT"].T
    return out


def kernel(input_seq, W0, U0, b0, W1, U1, b1, Wfc, bfc):
    nc = _get_compiled()
    return _run(nc, dict(input_seq=input_seq, W0=W0, U0=U0, b0=b0, W1=W1,
                         U1=U1, b1=b1, Wfc=Wfc, bfc=bfc))



# revision 5
# speedup vs baseline: 2.9007x; 2.9007x over previous
"""Two-layer LSTM + FC head on 8 NeuronCores — v3.

On top of v2's interleaved layers (layer 1 lags layer 0 by one chunk so
each layer's gate chain hides under the other's matmul block):

  - xw pre-fill via identity matmul: each step's z accumulation starts
    with an identity-weight matmul that copies xw_t (bf16) into PSUM
    (start=True), then the 16 U-tile matmuls accumulate on top. This
    removes the separate DVE add and one cross-engine hop; the sigmoid
    reads PSUM directly (cheaper ACT fixed cost).
  - gate chain: ACT sigmoid(i,f,o) PSUM->SBUF, then 4 DVE ops
    (ig = i*g with g straight from PSUM, c = f*c, c += ig, h = o*c).
  - PE warm-up burst before the loop (HAM clock gate: PE runs at 1.2
    GHz until ~3.4us of sustained busy; warm = 2.4 GHz).
  - For_i uses staggered_reset + hint_engines to avoid the ~2-6us
    all-engine back-edge barrier + IRAM-miss stall per chunk.
  - xw0t/xw1t stored bf16 (matmul rhs dtype must match the bf16
    identity weights; also halves their SBUF footprint).
"""

import os
import numpy as np
from contextlib import ExitStack

os.environ.setdefault("MYCRO_LOCAL_CACHE", "1")

B, T, I, H, O = 32, 2048, 128, 256, 128
NCORES = 8
BL = B // NCORES          # 4 batch elements per core
CHUNK = 64                # timesteps per loop body
G4 = 4 * H                # 1024 gate columns
NM = G4 // 128            # 8 gate chunks of 128
KT = H // 128             # 2 contraction tiles

MM_BF16 = True            # matmul operands in bf16

_cache = {}


def _np_mmdt():
    if MM_BF16:
        import ml_dtypes
        return ml_dtypes.bfloat16
    return np.float32


def _build(tiny=False, mm_bf16=None, repeat=None):
    import concourse.bacc as bacc
    import concourse.bass as bass
    import concourse.tile as tile
    import concourse.mybir as mybir

    if repeat is None:
        repeat = int(os.environ.get("BENCH_REPEAT", "1"))
    if mm_bf16 is None:
        mm_bf16 = MM_BF16
    f32 = mybir.dt.float32
    mdt = mybir.dt.bfloat16 if mm_bf16 else f32
    AF = mybir.ActivationFunctionType
    ALU = mybir.AluOpType
    ET = mybir.EngineType
    mode = os.environ.get("V3_MODE", "full")  # full | nochain

    nc = bacc.Bacc("TRN2", target_bir_lowering=False, debug=False,
                   num_devices=NCORES)

    xprep_d = nc.declare_dram_parameter("xprep", [I, BL, T], mdt, isOutput=False)
    w0_d = nc.declare_dram_parameter("w0", [I, G4], mdt, isOutput=False)
    u0_d = nc.declare_dram_parameter("u0", [H, G4], mdt, isOutput=False)
    w1_d = nc.declare_dram_parameter("w1", [H, G4], mdt, isOutput=False)
    u1_d = nc.declare_dram_parameter("u1", [H, G4], mdt, isOutput=False)
    wfc_d = nc.declare_dram_parameter("wfc", [H, O], mdt, isOutput=False)
    ident_d = nc.declare_dram_parameter("ident", [128, 128], mdt, isOutput=False)
    b0t_d = nc.declare_dram_parameter("b0t", [128, NM], f32, isOutput=False)
    b1t_d = nc.declare_dram_parameter("b1t", [128, NM], f32, isOutput=False)
    bfct_d = nc.declare_dram_parameter("bfct", [128, 1], f32, isOutput=False)
    out_d = nc.declare_dram_parameter("outT", [O, BL], f32, isOutput=True)

    with tile.TileContext(nc) as tc, ExitStack() as ctx:
        if tiny:
            pool = ctx.enter_context(tc.tile_pool(name="tp", bufs=1))
            t1 = pool.tile([128, BL], mdt, tag="t1")
            t2 = pool.tile([128, BL], f32, tag="t2")
            nc.sync.dma_start(t1[:, :], xprep_d[:, :, 0])
            nc.vector.tensor_copy(t2[:, :], t1[:, :])
            nc.sync.dma_start(out_d[:, :], t2[:, :])
            nc.compile()
            return nc

        const = ctx.enter_context(tc.tile_pool(name="const", bufs=1))
        work = ctx.enter_context(tc.tile_pool(name="work", bufs=3))
        psum = ctx.enter_context(tc.tile_pool(name="psum", bufs=2, space="PSUM"))

        # Persistent SBUF residents.
        xall = const.tile([128, BL * T], mdt, tag="xall")       # col = b*T + t
        w0 = const.tile([128, G4], mdt, tag="w0")
        u0 = [const.tile([128, G4], mdt, tag=f"u0_{k}", name=f"u0_{k}")
              for k in range(KT)]
        w1 = [const.tile([128, G4], mdt, tag=f"w1_{k}", name=f"w1_{k}")
              for k in range(KT)]
        u1 = [const.tile([128, G4], mdt, tag=f"u1_{k}", name=f"u1_{k}")
              for k in range(KT)]
        wf = [const.tile([128, O], mdt, tag=f"wf_{k}", name=f"wf_{k}")
              for k in range(KT)]
        ident = const.tile([128, 128], mdt, tag="ident")
        b0t = const.tile([128, NM], f32, tag="b0t")
        b1t = const.tile([128, NM], f32, tag="b1t")
        bfct = const.tile([128, 1], f32, tag="bfct")
        # chunk buffers: col = k*(BL*CHUNK) + b*CHUNK + t   (h0t)
        #                col = m*(BL*CHUNK) + b*CHUNK + t   (xw0t/xw1t)
        h0t = const.tile([128, KT * BL * CHUNK], mdt, tag="h0t")
        xw0t = const.tile([128, NM * BL * CHUNK], mdt, tag="xw0t")
        xw1t = const.tile([128, NM * BL * CHUNK], mdt, tag="xw1t")
        # recurrent state, col = k*BL + b
        c0 = const.tile([128, KT * BL], f32, tag="c0")
        c1 = const.tile([128, KT * BL], f32, tag="c1")
        h1 = const.tile([128, KT * BL], mdt, tag="h1")

        nc.sync.dma_start(xall[:, :].rearrange("p (b t) -> p b t", b=BL),
                          xprep_d[:, :, :])
        nc.sync.dma_start(w0[:, :], w0_d[:, :])
        for k in range(KT):
            sl = slice(k * 128, (k + 1) * 128)
            nc.sync.dma_start(u0[k][:, :], u0_d[sl, :])
            nc.sync.dma_start(w1[k][:, :], w1_d[sl, :])
            nc.sync.dma_start(u1[k][:, :], u1_d[sl, :])
            nc.sync.dma_start(wf[k][:, :], wfc_d[sl, :])
        nc.sync.dma_start(ident[:, :], ident_d[:, :])
        nc.sync.dma_start(b0t[:, :], b0t_d[:, :])
        nc.sync.dma_start(b1t[:, :], b1t_d[:, :])
        nc.sync.dma_start(bfct[:, :], bfct_d[:, :])

        nc.vector.memset(h0t[:, :], 0.0)
        nc.vector.memset(xw1t[:, :], 0.0)
        nc.vector.memset(c0[:, :], 0.0)
        nc.vector.memset(c1[:, :], 0.0)
        nc.vector.memset(h1[:, :], 0.0)

        # PE warm-up burst: ~40 dense N=512 matmuls (~8us) to flip the HAM
        # clock gate to 8/8 before the latency-sensitive stream starts.
        psw = psum.tile([128, 512], f32, tag="psx")
        for _ in range(40):
            nc.tensor.matmul(psw[:, :], lhsT=w0[:, 0:128], rhs=xall[:, 0:512],
                             start=True, stop=True)

        S = KT * BL

        def lstm_step(tl, uw, xwt, cst, h_rhs_fn, h_out_ap, lab):
            """One recurrence step. h_rhs_fn(k)->[128,BL] prev-h AP,
            h_out_ap: target AP for the new h."""
            zp = psum.tile([128, NM * BL], f32, tag=f"zp{lab}")
            xw_ap = xwt[:, :].rearrange("p (m b t) -> p m b t",
                                        m=NM, b=BL)[:, :, :, tl]
            # z := xw_t  (identity matmul pre-fill, clears has_written)
            nc.tensor.matmul(zp[:, :].rearrange("p (m b) -> p m b", m=NM),
                             lhsT=ident[:, :], rhs=xw_ap,
                             start=True, stop=False)
            # z += U^T h
            for m in range(NM):
                msl = slice(m * 128, (m + 1) * 128)
                for k in range(KT):
                    nc.tensor.matmul(zp[:, m * BL:(m + 1) * BL],
                                     lhsT=uw[k][:, msl], rhs=h_rhs_fn(k),
                                     start=False,
                                     stop=(m == NM - 1 and k == KT - 1))
            if mode == "nochain":
                return
            # gate cols (permuted): i 0:S, f S:2S, o 2S:3S, g 3S:4S
            zs = work.tile([128, 3 * S], f32, tag=f"zs{lab}")
            nc.scalar.activation(zs[:, :], zp[:, 0:3 * S], AF.Sigmoid)
            ig = work.tile([128, S], f32, tag=f"ig{lab}")
            nc.vector.tensor_tensor(ig[:, :], zs[:, 0:S],
                                    zp[:, 3 * S:4 * S], ALU.mult)
            nc.vector.tensor_tensor(cst[:, :], zs[:, S:2 * S],
                                    cst[:, :], ALU.mult)
            nc.vector.tensor_tensor(cst[:, :], cst[:, :], ig[:, :], ALU.add)
            nc.vector.tensor_tensor(
                h_out_ap,
                zs[:, 2 * S:3 * S].rearrange("p (k b) -> p k b", k=KT),
                cst[:, :].rearrange("p (k b) -> p k b", k=KT), ALU.mult)

        h0t_4d = h0t[:, :].rearrange("p (k b t) -> p k b t", k=KT, b=BL)
        h1_3d = h1[:, :].rearrange("p (k b) -> p k b", k=KT)

        def l1_step(tl):
            lstm_step(tl, u1, xw1t, c1,
                      lambda k: h1[:, k * BL:(k + 1) * BL], h1_3d, "1")

        rep_ctx = tc.For_i(0, repeat, 1) if repeat > 1 else None
        if rep_ctx is not None:
            rep_ctx.__enter__()
        with tc.For_i(0, T, CHUNK, staggered_reset=True,
                      hint_engines=(ET.PE, ET.DVE, ET.Activation)) as iv:
            # stage this chunk's x columns (only dynamic access in the body)
            xq = work.tile([128, BL * CHUNK], mdt, tag="xq")
            nc.vector.tensor_copy(
                xq[:, :].rearrange("p (b t) -> p b t", b=BL),
                xall[:, :].rearrange("p (b t) -> p b t",
                                     b=BL)[:, :, bass.ds(iv, CHUNK)])
            # xw0 = x @ W0 + b0 for the chunk
            for m in range(NM):
                msl = slice(m * 128, (m + 1) * 128)
                csl = slice(m * BL * CHUNK, (m + 1) * BL * CHUNK)
                psx = psum.tile([128, BL * CHUNK], f32, tag="psx")
                nc.tensor.matmul(psx[:, :], lhsT=w0[:, msl], rhs=xq[:, :],
                                 start=True, stop=True)
                nc.scalar.activation(xw0t[:, csl], psx[:, :], AF.Identity,
                                     bias=b0t[:, m:m + 1])
            # interleaved: layer-0 recurrence on this chunk, layer-1 on the
            # previous chunk (chunk 0: xw1t==0 and zero state -> exact no-op)
            for tl in range(CHUNK):
                pv = (tl - 1) % CHUNK
                lstm_step(
                    tl, u0, xw0t, c0,
                    lambda k: h0t_4d[:, k, :, pv],
                    h0t_4d[:, :, :, tl], "0")
                l1_step(tl)
            # xw1 = h0 @ W1 + b1 for this chunk (consumed next iteration)
            for m in range(NM):
                msl = slice(m * 128, (m + 1) * 128)
                csl = slice(m * BL * CHUNK, (m + 1) * BL * CHUNK)
                psx = psum.tile([128, BL * CHUNK], f32, tag="psx")
                for k in range(KT):
                    nc.tensor.matmul(
                        psx[:, :], lhsT=w1[k][:, msl],
                        rhs=h0t[:, k * BL * CHUNK:(k + 1) * BL * CHUNK],
                        start=(k == 0), stop=(k == KT - 1))
                nc.scalar.activation(xw1t[:, csl], psx[:, :], AF.Identity,
                                     bias=b1t[:, m:m + 1])
        if rep_ctx is not None:
            rep_ctx.__exit__(None, None, None)

        # layer-1 epilogue: final chunk
        for tl in range(CHUNK):
            l1_step(tl)

        # FC head: out^T = Wfc^T @ h1^T + bfc
        psf = psum.tile([128, BL], f32, tag="psf")
        for k in range(KT):
            nc.tensor.matmul(psf[:, :], lhsT=wf[k][:, :],
                             rhs=h1[:, k * BL:(k + 1) * BL],
                             start=(k == 0), stop=(k == KT - 1))
        oT = work.tile([128, BL], f32, tag="oT")
        nc.scalar.activation(oT[:, :], psf[:, :], AF.Identity,
                             bias=bfct[:, 0:1])
        nc.sync.dma_start(out_d[:, :], oT[:, :])

    nc.compile()
    return nc


def _get_compiled():
    if "main" not in _cache:
        _cache["main"] = _build()
    return _cache["main"]


def _in_maps(input_seq, W0, U0, b0, W1, U1, b1, Wfc, bfc):
    mdt = _np_mmdt()
    x = np.asarray(input_seq, dtype=np.float32)
    # reorder gate blocks (i,f,g,o) -> (i,f,o,g) so one sigmoid instr
    # covers the first three
    perm = np.concatenate([np.arange(0, 2 * H),
                           np.arange(3 * H, 4 * H),
                           np.arange(2 * H, 3 * H)])

    def gp(w):
        return np.ascontiguousarray(
            np.asarray(w, np.float32)[..., perm].astype(mdt))

    shared = {
        "w0": gp(W0),
        "u0": gp(U0),
        "w1": gp(W1),
        "u1": gp(U1),
        "wfc": np.ascontiguousarray(np.asarray(Wfc, np.float32).astype(mdt)),
        "ident": np.ascontiguousarray(np.eye(128, dtype=np.float32).astype(mdt)),
        "b0t": np.ascontiguousarray(
            np.asarray(b0, np.float32)[perm].reshape(NM, 128).T),
        "b1t": np.ascontiguousarray(
            np.asarray(b1, np.float32)[perm].reshape(NM, 128).T),
        "bfct": np.ascontiguousarray(np.asarray(bfc, np.float32).reshape(1, 128).T),
    }
    in_maps = []
    for c in range(NCORES):
        xs = x[c * BL:(c + 1) * BL]                       # [BL, T, I]
        xp = np.ascontiguousarray(xs.transpose(2, 0, 1).astype(mdt))
        m = dict(shared)
        m["xprep"] = xp
        in_maps.append(m)
    return in_maps


def _run(nc, inputs):
    from concourse.bass_utils import run_bass_kernel_spmd
    in_maps = _in_maps(**inputs)
    res = run_bass_kernel_spmd(nc, in_maps, list(range(NCORES)))
    out = np.empty((B, 1, O), np.float32)
    for c in range(NCORES):
        out[c * BL:(c + 1) * BL, 0, :] = res.results[c]["outT"].T
    return out


def kernel(input_seq, W0, U0, b0, W1, U1, b1, Wfc, bfc):
    nc = _get_compiled()
    return _run(nc, dict(input_seq=input_seq, W0=W0, U0=U0, b0=b0, W1=W1,
                         U1=U1, b1=b1, Wfc=Wfc, bfc=bfc))
